# revision 6
# baseline (speedup 1.0000x reference)
"""Trainium2 kernel for nn_HadamardRotation: y = x @ H, H = 4096x4096 Walsh-Hadamard.

Strategy
--------
H4096 = H64 (x) H64 (Kronecker). Writing d = 64*hi + lo, e = 64*hi' + lo':

    y[r, e] = sum_{hi,lo} H64[lo,lo'] * H64[hi,hi'] * x[r, d]

Two matmul stages with 128-wide contraction (block-diagonal I2 (x) H64
weights), separated by an on-chip "corner turn", all operating in the
transposed domain (d on partitions, rows on the free axis). The corner
turn is done with DVE InstStreamTranspose (32x32 partition<->free block
transposes), NOT DMA: SBUF->SBUF DMA is limited to ~40 GB/s aggregate on
this part, which made a DMA-based turn the bottleneck.

Stage A emits psum partitions ordered m = 32*t + c (t = 2*nu + mu the
32-block index, c = lo'>>1 within the block); the stream transpose then
swaps (c <-> a) within each 32-block, yielding partitions q = 32*t + a =
64*nu + 32*mu + a with chunk c on the free axis - exactly stage B's
contraction layout.

FLOPs: 2 * 128/4096 of the naive matmul = 16x reduction.

Data parallel over 8 cores: rows sharded 16384 -> 8 x 2048, weights
replicated. Host does the cheap layout permutes / final cast (not timed).

Per-core layouts (R = 2048 rows, N = 512 slab, SG = 4 slab groups):
  xt DRAM in  [SG*8, 128, IB*N]: mirrors the SBUF xg tiles so each input
     DMA is 128 contiguous 4KB descriptors.
     xt[sg*8+g, q, j*N+rr] = x[r=sg*N+rr, d=128*(4g+j)+q]
  B1 (128,128): B1[64*mu+lo, 32*(2nu+mu)+c] = H64[lo, 2c+nu]
  B2 (128,128): B2[64*nu+32*mu+a, 2*hi'+nu] = H64[2*a+mu, hi']
  stage A (chunk a=4g+j): u[m, a, rr] = sum_k B1[k, m] xg[k, j, rr]
  turn: v[32t+a, c, rr] = u[32t+c, a, rr]   (DVE stream transpose)
  stage B (chunk c): yb[m2, rr] = sum_q B2[q, m2] v[q, c, rr]
      => yb[2*hi'+nu] = y[rr, 64*hi' + 2*c + nu]  (bf16)
  Y DRAM out [SG*8, 128, OB*N]: mirrors the SBUF yb tiles; host
     unscrambles + casts to f32.
"""

import math
import numpy as np
import ml_dtypes

import concourse.bass as bass
import concourse.mybir as mybir
import concourse.tile as tile
from concourse import bacc
from concourse.bass_utils import run_bass_kernel_spmd

N_CORES = 8
DIM = 4096
R_TOTAL = 4 * 4096          # rows after flattening (4, 4096, DIM)
R = R_TOTAL // N_CORES      # rows per core
N = 512                     # matmul free-dim slab (one PSUM bank of fp32)
MODE = "bf16"               # storage dtype for x/intermediate/output

# tuning knobs
CFG = dict(
    IB=4,              # chunks per input DMA / xg tile
    OB=4,              # chunks per output DMA / yb tile
    ucopy_engs="vector,scalar",  # stage-A psum->sbuf copy engines
    ycopy_engs="scalar,vector",  # stage-B psum->sbuf copy engines
    in_eng="sync",
    out_eng="sync",
    turn_splits=1,     # stream-transpose instructions per slab group
    xbufs=2, ubufs=2, vbufs=2, ybufs=2,
)


def _walsh_hadamard64():
    h = np.array([[1.0]], dtype=np.float64)
    while h.shape[0] < 64:
        h = np.block([[h, h], [h, -h]]) / math.sqrt(2.0)
    return h.astype(np.float32)


def _build_weights(H64):
    # B1[64*mu+lo, 32*(2*nu+mu')+c] = H64[lo, 2c+nu] if mu'==mu else 0
    B1 = np.zeros((128, 128), dtype=np.float32)
    b1v = B1.reshape(2, 64, 2, 2, 32)       # [mu, lo, nu, mu', c]
    for mu in range(2):
        for nu in range(2):
            b1v[mu, :, nu, mu, :] = H64[:, nu::2]
    # B2[64*nu+32*mu+a, 2*hi'+nu'] = H64[2a+mu, hi'] if nu'==nu else 0
    B2 = np.zeros((128, 128), dtype=np.float32)
    b2v = B2.reshape(2, 2, 32, 64, 2)       # [nu, mu, a, hi', nu']
    for nu in range(2):
        for mu in range(2):
            b2v[nu, mu, :, :, nu] = H64[mu::2, :]
    return B1, B2


_NC_CACHE = {}


def _build_bass(cfg=None):
    cfg = dict(CFG, **(cfg or {}))
    key = tuple(sorted(cfg.items()))
    if key in _NC_CACHE:
        return _NC_CACHE[key]

    f32 = mybir.dt.float32
    bf16 = mybir.dt.bfloat16

    IB, OB = cfg["IB"], cfg["OB"]
    SG = R // N                 # slab groups
    NG = 32 // IB               # input chunk groups per slab group
    NCB = 32 // OB              # output chunk batches per slab group
    TSPL = cfg["turn_splits"]

    nc = bacc.Bacc("TRN2", target_bir_lowering=False, debug=False,
                   num_devices=N_CORES)
    xt_d = nc.dram_tensor("xt", [SG * NG, 128, IB * N], bf16,
                          kind="ExternalInput")
    B1_d = nc.dram_tensor("B1", [128, 128], bf16, kind="ExternalInput")
    B2_d = nc.dram_tensor("B2", [128, 128], bf16, kind="ExternalInput")
    Y_d = nc.dram_tensor("Y", [SG * NCB, 128, OB * N], bf16,
                         kind="ExternalOutput")

    with tile.TileContext(nc) as tc:
        with (
            tc.tile_pool(name="wpool", bufs=1) as wpool,
            tc.tile_pool(name="xpool", bufs=cfg["xbufs"]) as xpool,
            tc.tile_pool(name="upool", bufs=cfg["ubufs"]) as upool,
            tc.tile_pool(name="vpool", bufs=cfg["vbufs"]) as vpool,
            tc.tile_pool(name="ypool", bufs=cfg["ybufs"]) as ypool,
            tc.tile_pool(name="psA", bufs=4, space="PSUM") as psA,
            tc.tile_pool(name="psB", bufs=4, space="PSUM") as psB,
        ):
            B1_sb = wpool.tile([128, 128], bf16)
            nc.sync.dma_start(B1_sb[:], B1_d[:])
            B2_sb = wpool.tile([128, 128], bf16)
            nc.sync.dma_start(B2_sb[:], B2_d[:])

            in_eng = getattr(nc, cfg["in_eng"])
            out_eng = getattr(nc, cfg["out_eng"])

            def eng_list(names):
                return [getattr(nc, nm.strip()) for nm in names.split(",")]

            ucopy_engs = eng_list(cfg["ucopy_engs"])
            ycopy_engs = eng_list(cfg["ycopy_engs"])

            def copy(engs, i, dst, src):
                e = engs[i % len(engs)]
                if e is nc.scalar:
                    nc.scalar.copy(dst, src)
                else:
                    e.tensor_copy(dst, src)

            def phaseA(sg):
                u_all = upool.tile([128, 32, N], bf16)
                for g in range(NG):
                    xg = xpool.tile([128, IB, N], bf16)
                    in_eng.dma_start(xg[:], xt_d[sg * NG + g, :, :])
                    for j in range(IB):
                        a = IB * g + j
                        pu = psA.tile([128, N], f32)
                        nc.tensor.matmul(pu[:], B1_sb[:], xg[:, j, :],
                                         start=True, stop=True)
                        copy(ucopy_engs, a, u_all[:, a, :], pu[:])
                # corner turn: v[32t+a, c, rr] = u[32t+c, a, rr]
                v_all = vpool.tile([128, 32, N], bf16)
                ut, vt = u_all.tensor, v_all.tensor
                PU = u_all.ap[0][0]
                PV = v_all.ap[0][0]
                W = N // TSPL
                for s in range(TSPL):
                    in_ap = bass.AP(ut, s * W, [[PU, 128], [1, W], [N, 32]])
                    out_ap = bass.AP(vt, s * W, [[PV, 128], [1, W], [N, 32]])
                    nc.vector.transpose(out_ap, in_ap)
                return v_all

            def phaseB(sg, v_all):
                for cb in range(NCB):
                    yb = ypool.tile([128, OB, N], bf16)
                    for j in range(OB):
                        c = cb * OB + j
                        py = psB.tile([128, N], f32)
                        nc.tensor.matmul(py[:], B2_sb[:], v_all[:, c, :],
                                         start=True, stop=True)
                        copy(ycopy_engs, c, yb[:, j, :], py[:])
                    out_eng.dma_start(Y_d[sg * NCB + cb, :, :], yb[:])

            # software pipeline: emit stage A of slab-group sg+1 before
            # stage B of sg, so the PE never stalls on the corner turn.
            pending = None
            for sg in range(SG):
                v_all = phaseA(sg)
                if pending is not None:
                    phaseB(*pending)
                pending = (sg, v_all)
            phaseB(*pending)

    nc.compile()
    _NC_CACHE[key] = nc
    return nc


def _prep_inputs(x, H, cfg=None):
    cfg = dict(CFG, **(cfg or {}))
    IB = cfg["IB"]
    SG = R // N
    NG = 32 // IB
    H64 = (np.asarray(H, dtype=np.float32)[::64, ::64] * 8.0).astype(np.float32)
    B1, B2 = _build_weights(H64)
    B1 = B1.astype(ml_dtypes.bfloat16)
    B2 = B2.astype(ml_dtypes.bfloat16)
    xf = np.asarray(x, dtype=np.float32).reshape(R_TOTAL, DIM)
    in_maps = []
    for i in range(N_CORES):
        shard = xf[i * R:(i + 1) * R]                     # (R, DIM)
        # [sg, rr, a, q] -> [sg, g, q, j, rr]
        xt = shard.reshape(SG, N, 32, 128)
        xt = xt.transpose(0, 2, 3, 1).reshape(SG, NG, IB, 128, N)
        xt = np.ascontiguousarray(xt.transpose(0, 1, 3, 2, 4),
                                  dtype=ml_dtypes.bfloat16)
        xt = xt.reshape(SG * NG, 128, IB * N)
        in_maps.append({"xt": xt, "B1": B1, "B2": B2})
    return in_maps


def _unscramble(results, cfg=None):
    cfg = dict(CFG, **(cfg or {}))
    OB = cfg["OB"]
    SG = R // N
    NCB = 32 // OB
    outs = []
    for i in range(N_CORES):
        Y = results[i]["Y"]      # [SG*NCB, 128, OB*N] bf16
        # [sg, cb, (hi', nu), j, rr] -> [sg, rr, hi', (cb, j, nu)]
        y = np.asarray(Y, dtype=np.float32).reshape(SG, NCB, 64, 2, OB, N)
        y = y.transpose(0, 5, 2, 1, 4, 3).reshape(R, DIM)
        outs.append(y)
    return np.concatenate(outs, axis=0).reshape(4, 4096, DIM).astype(np.float32)


def kernel(x, H, _trace=False, _cfg=None):
    nc = _build_bass(_cfg)
    in_maps = _prep_inputs(x, H, _cfg)
    res = run_bass_kernel_spmd(nc, in_maps, core_ids=list(range(N_CORES)),
                               trace=_trace)
    out = _unscramble(res.results, _cfg)
    if _trace:
        return out, res
    return out


# revision 7
# speedup vs baseline: 1.0305x; 1.0305x over previous
"""Trainium2 kernel for nn_HadamardRotation: y = x @ H, H = 4096x4096 Walsh-Hadamard.

Strategy
--------
H4096 = H64 (x) H64 (Kronecker). Writing d = 64*hi + lo, e = 64*hi' + lo':

    y[r, e] = sum_{hi,lo} H64[lo,lo'] * H64[hi,hi'] * x[r, d]

Two matmul stages with 128-wide contraction (block-diagonal I2 (x) H64
weights), separated by an on-chip "corner turn" (SBUF->SBUF DMA partition
shuffle), all operating in the transposed domain (d on partitions, rows on
the free axis).

The corner turn dominates; it is tuned for the DMA engines' per-descriptor
cost: the whole per-core row range (L=2048) is kept in one SBUF-resident
intermediate so every turn descriptor is a full 4KB line, and the 32 turn
DMAs are spread over multiple queues so their descriptors hit all 16 DMA
engines. Input and output DRAM tensors exactly mirror the SBUF tiles
(contiguous 4-8KB per partition per DMA).

FLOPs: 2 * 128/4096 of the naive matmul = 16x reduction.

Data parallel over 8 cores: rows sharded 16384 -> 8 x 2048, weights
replicated. Host does the layout permutes / final f32 cast (not timed).

Per-core layouts (R = 2048 rows = L, N = 512 matmul slab):
  xt DRAM in  [16, 128, IB*L]: xt[g, q, j*L+rr] = x[rr, 128*(IB*g+j)+q]
  B1 (128,128): B1[64*mu+lo, 32*(2nu+mu)+c] = H64[lo, 2c+nu]
  B2 (128,128): B2[64*nu+32*mu+a, 2*hi'+nu] = H64[2*a+mu, hi']
  stage A (chunk a): u[m, a, rr] = sum_k B1[k, m] xg[k, j, rr]
      => u[32*(2nu+mu) + c, a] holds (hi = 2a+mu, lo' = 2c+nu)
  corner turn (chunk c): vc[32t+a, rr] = u[32t+c, a, rr]
  stage B (chunk c): yb[m2, rr] = sum_q B2[q, m2] vc[q, rr]
      => yb[2*hi'+nu] = y[rr, 64*hi' + 2*c + nu]  (bf16)
  Y DRAM out [32/OB, 128, OB*L]: mirrors the SBUF yb tiles; host
     unscrambles + casts to f32.
"""

import math
import numpy as np
import ml_dtypes

import concourse.bass as bass
import concourse.mybir as mybir
import concourse.tile as tile
from concourse import bacc
from concourse.bass_utils import run_bass_kernel_spmd

N_CORES = 8
DIM = 4096
R_TOTAL = 4 * 4096          # rows after flattening (4, 4096, DIM)
R = R_TOTAL // N_CORES      # rows per core
L = R                       # all rows resident: 4KB turn descriptors
N = 512                     # matmul free-dim slab (one PSUM bank of fp32)
TS = L // N                 # matmul slabs per chunk
MODE = "bf16"               # storage dtype for x/intermediate/output

# tuning knobs
CFG = dict(
    IB=2,              # chunks per input DMA / xg tile
    OB=2,              # chunks per output DMA / yb tile
    ucopy_engs="vector,scalar",  # stage-A psum->sbuf copy engines
    ycopy_engs="scalar,vector",  # stage-B psum->sbuf copy engines
    in_eng="sync",
    out_eng="gpsimd",
    turn_engs="sync,gpsimd",     # round-robin queues for turn DMAs
    xbufs=2, vbufs=4, ybufs=2,
)


def _walsh_hadamard64():
    h = np.array([[1.0]], dtype=np.float64)
    while h.shape[0] < 64:
        h = np.block([[h, h], [h, -h]]) / math.sqrt(2.0)
    return h.astype(np.float32)


def _build_weights(H64):
    # B1[64*mu+lo, 32*(2*nu+mu')+c] = H64[lo, 2c+nu] if mu'==mu else 0
    B1 = np.zeros((128, 128), dtype=np.float32)
    b1v = B1.reshape(2, 64, 2, 2, 32)       # [mu, lo, nu, mu', c]
    for mu in range(2):
        for nu in range(2):
            b1v[mu, :, nu, mu, :] = H64[:, nu::2]
    # B2[64*nu+32*mu+a, 2*hi'+nu'] = H64[2a+mu, hi'] if nu'==nu else 0
    B2 = np.zeros((128, 128), dtype=np.float32)
    b2v = B2.reshape(2, 2, 32, 64, 2)       # [nu, mu, a, hi', nu']
    for nu in range(2):
        for mu in range(2):
            b2v[nu, mu, :, :, nu] = H64[mu::2, :]
    return B1, B2


_NC_CACHE = {}


def _build_bass(cfg=None):
    cfg = dict(CFG, **(cfg or {}))
    key = tuple(sorted(cfg.items()))
    if key in _NC_CACHE:
        return _NC_CACHE[key]

    f32 = mybir.dt.float32
    bf16 = mybir.dt.bfloat16

    IB, OB = cfg["IB"], cfg["OB"]
    NG = 32 // IB               # input chunk groups
    NCB = 32 // OB              # output chunk batches

    nc = bacc.Bacc("TRN2", target_bir_lowering=False, debug=False,
                   num_devices=N_CORES)
    xt_d = nc.dram_tensor("xt", [NG, 128, IB * L], bf16, kind="ExternalInput")
    B1_d = nc.dram_tensor("B1", [128, 128], bf16, kind="ExternalInput")
    B2_d = nc.dram_tensor("B2", [128, 128], bf16, kind="ExternalInput")
    Y_d = nc.dram_tensor("Y", [NCB, 128, OB * L], bf16, kind="ExternalOutput")

    with tile.TileContext(nc) as tc:
        with (
            tc.tile_pool(name="wpool", bufs=1) as wpool,
            tc.tile_pool(name="xpool", bufs=cfg["xbufs"]) as xpool,
            tc.tile_pool(name="upool", bufs=1) as upool,
            tc.tile_pool(name="vpool", bufs=cfg["vbufs"]) as vpool,
            tc.tile_pool(name="ypool", bufs=cfg["ybufs"]) as ypool,
            tc.tile_pool(name="psA", bufs=4, space="PSUM") as psA,
            tc.tile_pool(name="psB", bufs=4, space="PSUM") as psB,
        ):
            B1_sb = wpool.tile([128, 128], bf16)
            nc.sync.dma_start(B1_sb[:], B1_d[:])
            B2_sb = wpool.tile([128, 128], bf16)
            nc.sync.dma_start(B2_sb[:], B2_d[:])

            in_eng = getattr(nc, cfg["in_eng"])
            out_eng = getattr(nc, cfg["out_eng"])

            def eng_list(names):
                return [getattr(nc, nm.strip()) for nm in names.split(",")]

            ucopy_engs = eng_list(cfg["ucopy_engs"])
            ycopy_engs = eng_list(cfg["ycopy_engs"])
            turn_engs = eng_list(cfg["turn_engs"])

            def copy(engs, i, dst, src):
                e = engs[i % len(engs)]
                if e is nc.scalar:
                    nc.scalar.copy(dst, src)
                else:
                    e.tensor_copy(dst, src)

            u_all = upool.tile([128, 32, L], bf16)
            ut = u_all.tensor
            PU = u_all.ap[0][0]  # partition stride in elements

            # stage A
            for g in range(NG):
                xg = xpool.tile([128, IB, L], bf16)
                in_eng.dma_start(xg[:], xt_d[g, :, :])
                for j in range(IB):
                    a = IB * g + j
                    for ts in range(TS):
                        pu = psA.tile([128, N], f32)
                        nc.tensor.matmul(pu[:], B1_sb[:],
                                         xg[:, j, ts * N:(ts + 1) * N],
                                         start=True, stop=True)
                        copy(ucopy_engs, a * TS + ts,
                             u_all[:, a, ts * N:(ts + 1) * N], pu[:])

            # corner turn + stage B
            for cb in range(NCB):
                yb = ypool.tile([128, OB, L], bf16)
                for j in range(OB):
                    c = cb * OB + j
                    vc = vpool.tile([128, L], bf16)
                    in_ap = bass.AP(ut, c * PU,
                                    [[32 * PU, 4], [L, 32], [1, L]])
                    turn_engs[c % len(turn_engs)].dma_start(vc[:], in_ap)
                    for ts in range(TS):
                        py = psB.tile([128, N], f32)
                        nc.tensor.matmul(py[:], B2_sb[:],
                                         vc[:, ts * N:(ts + 1) * N],
                                         start=True, stop=True)
                        copy(ycopy_engs, c * TS + ts,
                             yb[:, j, ts * N:(ts + 1) * N], py[:])
                out_eng.dma_start(Y_d[cb, :, :], yb[:])

    nc.compile()
    _NC_CACHE[key] = nc
    return nc


def _prep_inputs(x, H, cfg=None):
    cfg = dict(CFG, **(cfg or {}))
    IB = cfg["IB"]
    NG = 32 // IB
    H64 = (np.asarray(H, dtype=np.float32)[::64, ::64] * 8.0).astype(np.float32)
    B1, B2 = _build_weights(H64)
    B1 = B1.astype(ml_dtypes.bfloat16)
    B2 = B2.astype(ml_dtypes.bfloat16)
    xf = np.asarray(x, dtype=np.float32).reshape(R_TOTAL, DIM)
    in_maps = []
    for i in range(N_CORES):
        shard = xf[i * R:(i + 1) * R]                     # (R, DIM)
        # [rr, a, q] -> [g, q, j, rr]
        xt = shard.reshape(L, 32, 128).transpose(1, 2, 0)   # [a, q, rr]
        xt = xt.reshape(NG, IB, 128, L).transpose(0, 2, 1, 3)
        xt = np.ascontiguousarray(xt, dtype=ml_dtypes.bfloat16)
        xt = xt.reshape(NG, 128, IB * L)
        in_maps.append({"xt": xt, "B1": B1, "B2": B2})
    return in_maps


def _unscramble(results, cfg=None):
    cfg = dict(CFG, **(cfg or {}))
    OB = cfg["OB"]
    NCB = 32 // OB
    outs = []
    for i in range(N_CORES):
        Y = results[i]["Y"]      # [NCB, 128, OB*L] bf16
        # [cb, (hi', nu), j, rr] -> [rr, hi', (cb, j, nu)]
        y = np.asarray(Y, dtype=np.float32).reshape(NCB, 64, 2, OB, L)
        y = y.transpose(4, 1, 0, 3, 2).reshape(R, DIM)
        outs.append(y)
    return np.concatenate(outs, axis=0).reshape(4, 4096, DIM).astype(np.float32)


def kernel(x, H, _trace=False, _cfg=None):
    nc = _build_bass(_cfg)
    in_maps = _prep_inputs(x, H, _cfg)
    res = run_bass_kernel_spmd(nc, in_maps, core_ids=list(range(N_CORES)),
                               trace=_trace)
    out = _unscramble(res.results, _cfg)
    if _trace:
        return out, res
    return out


# revision 8
# speedup vs baseline: 1.1965x; 1.1611x over previous
"""Trainium2 kernel for nn_HadamardRotation: y = x @ H, H = 4096x4096 Walsh-Hadamard.

Strategy
--------
H4096 = H64 (x) H64 (Kronecker). Writing d = 64*hi + lo, e = 64*hi' + lo':

    y[r, e] = sum_{hi,lo} H64[lo,lo'] * H64[hi,hi'] * x[r, d]

Two matmul stages with 128-wide contraction (block-diagonal I2 (x) H64
weights), separated by an on-chip "corner turn" (SBUF->SBUF DMA partition
shuffle), all operating in the transposed domain (d on partitions, rows on
the free axis).

The corner turn dominates; it is tuned for the DMA engines' per-descriptor
cost: the whole per-core row range (L=2048) is kept in one SBUF-resident
intermediate so every turn descriptor is a full 4KB line, and the 32 turn
DMAs are spread over multiple queues so their descriptors hit all 16 DMA
engines. Input and output DRAM tensors exactly mirror the SBUF tiles
(contiguous 4-8KB per partition per DMA).

FLOPs: 2 * 128/4096 of the naive matmul = 16x reduction.

Data parallel over 8 cores: rows sharded 16384 -> 8 x 2048, weights
replicated. Host does the layout permutes / final f32 cast (not timed).

Per-core layouts (R = 2048 rows = L, N = 512 matmul slab):
  xt DRAM in  [16, 128, IB*L]: xt[g, q, j*L+rr] = x[rr, 128*(IB*g+j)+q]
  B1 (128,128): B1[64*mu+lo, 32*(2nu+mu)+c] = H64[lo, 2c+nu]
  B2 (128,128): B2[64*nu+32*mu+a, 2*hi'+nu] = H64[2*a+mu, hi']
  stage A (chunk a): u[m, a, rr] = sum_k B1[k, m] xg[k, j, rr]
      => u[32*(2nu+mu) + c, a] holds (hi = 2a+mu, lo' = 2c+nu)
  corner turn (chunk c): vc[32t+a, rr] = u[32t+c, a, rr]
  stage B (chunk c): yb[m2, rr] = sum_q B2[q, m2] vc[q, rr]
      => yb[2*hi'+nu] = y[rr, 64*hi' + 2*c + nu]  (bf16)
  Y DRAM out [32/OB, 128, OB*L]: mirrors the SBUF yb tiles; host
     unscrambles + casts to f32.
"""

import math
import numpy as np
import ml_dtypes

import concourse.bass as bass
import concourse.mybir as mybir
import concourse.tile as tile
from concourse import bacc
from concourse.bass_utils import run_bass_kernel_spmd

N_CORES = 8
DIM = 4096
R_TOTAL = 4 * 4096          # rows after flattening (4, 4096, DIM)
R = R_TOTAL // N_CORES      # rows per core
L = R                       # all rows resident: 4KB turn descriptors
N = 512                     # matmul free-dim slab (one PSUM bank of fp32)
TS = L // N                 # matmul slabs per chunk
MODE = "bf16"               # storage dtype for x/intermediate/output

# tuning knobs
CFG = dict(
    IB=2,              # chunks per input DMA / xg tile
    OB=2,              # chunks per output DMA / yb tile
    ucopy_engs="vector,scalar",  # stage-A psum->sbuf copy engines
    ycopy_engs="scalar,vector",  # stage-B psum->sbuf copy engines
    in_eng="sync",
    out_eng="sync",
    turn_engs="gpsimd",          # dedicated queue so turns never queue
                                 # behind dependency-blocked out-DMAs
    xbufs=2, vbufs=6, ybufs=2,
)


def _walsh_hadamard64():
    h = np.array([[1.0]], dtype=np.float64)
    while h.shape[0] < 64:
        h = np.block([[h, h], [h, -h]]) / math.sqrt(2.0)
    return h.astype(np.float32)


def _build_weights(H64):
    # B1[64*mu+lo, 32*(2*nu+mu')+c] = H64[lo, 2c+nu] if mu'==mu else 0
    B1 = np.zeros((128, 128), dtype=np.float32)
    b1v = B1.reshape(2, 64, 2, 2, 32)       # [mu, lo, nu, mu', c]
    for mu in range(2):
        for nu in range(2):
            b1v[mu, :, nu, mu, :] = H64[:, nu::2]
    # B2[64*nu+32*mu+a, 2*hi'+nu'] = H64[2a+mu, hi'] if nu'==nu else 0
    B2 = np.zeros((128, 128), dtype=np.float32)
    b2v = B2.reshape(2, 2, 32, 64, 2)       # [nu, mu, a, hi', nu']
    for nu in range(2):
        for mu in range(2):
            b2v[nu, mu, :, :, nu] = H64[mu::2, :]
    return B1, B2


_NC_CACHE = {}


def _build_bass(cfg=None):
    cfg = dict(CFG, **(cfg or {}))
    key = tuple(sorted(cfg.items()))
    if key in _NC_CACHE:
        return _NC_CACHE[key]

    f32 = mybir.dt.float32
    bf16 = mybir.dt.bfloat16

    IB, OB = cfg["IB"], cfg["OB"]
    NG = 32 // IB               # input chunk groups
    NCB = 32 // OB              # output chunk batches

    nc = bacc.Bacc("TRN2", target_bir_lowering=False, debug=False,
                   num_devices=N_CORES)
    xt_d = nc.dram_tensor("xt", [NG, 128, IB * L], bf16, kind="ExternalInput")
    B1_d = nc.dram_tensor("B1", [128, 128], bf16, kind="ExternalInput")
    B2_d = nc.dram_tensor("B2", [128, 128], bf16, kind="ExternalInput")
    Y_d = nc.dram_tensor("Y", [NCB, 128, OB * L], bf16, kind="ExternalOutput")

    with tile.TileContext(nc) as tc:
        with (
            tc.tile_pool(name="wpool", bufs=1) as wpool,
            tc.tile_pool(name="xpool", bufs=cfg["xbufs"]) as xpool,
            tc.tile_pool(name="upool", bufs=1) as upool,
            tc.tile_pool(name="vpool", bufs=cfg["vbufs"]) as vpool,
            tc.tile_pool(name="ypool", bufs=cfg["ybufs"]) as ypool,
            tc.tile_pool(name="psA", bufs=4, space="PSUM") as psA,
            tc.tile_pool(name="psB", bufs=4, space="PSUM") as psB,
        ):
            B1_sb = wpool.tile([128, 128], bf16)
            nc.sync.dma_start(B1_sb[:], B1_d[:])
            B2_sb = wpool.tile([128, 128], bf16)
            nc.sync.dma_start(B2_sb[:], B2_d[:])

            in_eng = getattr(nc, cfg["in_eng"])
            out_eng = getattr(nc, cfg["out_eng"])

            def eng_list(names):
                return [getattr(nc, nm.strip()) for nm in names.split(",")]

            ucopy_engs = eng_list(cfg["ucopy_engs"])
            ycopy_engs = eng_list(cfg["ycopy_engs"])
            turn_engs = eng_list(cfg["turn_engs"])

            def copy(engs, i, dst, src):
                e = engs[i % len(engs)]
                if e is nc.scalar:
                    nc.scalar.copy(dst, src)
                else:
                    e.tensor_copy(dst, src)

            u_all = upool.tile([128, 32, L], bf16)
            ut = u_all.tensor
            PU = u_all.ap[0][0]  # partition stride in elements

            # stage A
            for g in range(NG):
                xg = xpool.tile([128, IB, L], bf16)
                in_eng.dma_start(xg[:], xt_d[g, :, :])
                for j in range(IB):
                    a = IB * g + j
                    for ts in range(TS):
                        pu = psA.tile([128, N], f32)
                        nc.tensor.matmul(pu[:], B1_sb[:],
                                         xg[:, j, ts * N:(ts + 1) * N],
                                         start=True, stop=True)
                        copy(ucopy_engs, a * TS + ts,
                             u_all[:, a, ts * N:(ts + 1) * N], pu[:])

            # corner turn + stage B
            for cb in range(NCB):
                yb = ypool.tile([128, OB, L], bf16)
                for j in range(OB):
                    c = cb * OB + j
                    vc = vpool.tile([128, L], bf16)
                    in_ap = bass.AP(ut, c * PU,
                                    [[32 * PU, 4], [L, 32], [1, L]])
                    turn_engs[c % len(turn_engs)].dma_start(vc[:], in_ap)
                    for ts in range(TS):
                        py = psB.tile([128, N], f32)
                        nc.tensor.matmul(py[:], B2_sb[:],
                                         vc[:, ts * N:(ts + 1) * N],
                                         start=True, stop=True)
                        copy(ycopy_engs, c * TS + ts,
                             yb[:, j, ts * N:(ts + 1) * N], py[:])
                out_eng.dma_start(Y_d[cb, :, :], yb[:])

    nc.compile()
    _NC_CACHE[key] = nc
    return nc


def _prep_inputs(x, H, cfg=None):
    cfg = dict(CFG, **(cfg or {}))
    IB = cfg["IB"]
    NG = 32 // IB
    H64 = (np.asarray(H, dtype=np.float32)[::64, ::64] * 8.0).astype(np.float32)
    B1, B2 = _build_weights(H64)
    B1 = B1.astype(ml_dtypes.bfloat16)
    B2 = B2.astype(ml_dtypes.bfloat16)
    xf = np.asarray(x, dtype=np.float32).reshape(R_TOTAL, DIM)
    in_maps = []
    for i in range(N_CORES):
        shard = xf[i * R:(i + 1) * R]                     # (R, DIM)
        # [rr, a, q] -> [g, q, j, rr]
        xt = shard.reshape(L, 32, 128).transpose(1, 2, 0)   # [a, q, rr]
        xt = xt.reshape(NG, IB, 128, L).transpose(0, 2, 1, 3)
        xt = np.ascontiguousarray(xt, dtype=ml_dtypes.bfloat16)
        xt = xt.reshape(NG, 128, IB * L)
        in_maps.append({"xt": xt, "B1": B1, "B2": B2})
    return in_maps


def _unscramble(results, cfg=None):
    cfg = dict(CFG, **(cfg or {}))
    OB = cfg["OB"]
    NCB = 32 // OB
    outs = []
    for i in range(N_CORES):
        Y = results[i]["Y"]      # [NCB, 128, OB*L] bf16
        # [cb, (hi', nu), j, rr] -> [rr, hi', (cb, j, nu)]
        y = np.asarray(Y, dtype=np.float32).reshape(NCB, 64, 2, OB, L)
        y = y.transpose(4, 1, 0, 3, 2).reshape(R, DIM)
        outs.append(y)
    return np.concatenate(outs, axis=0).reshape(4, 4096, DIM).astype(np.float32)


def kernel(x, H, _trace=False, _cfg=None):
    nc = _build_bass(_cfg)
    in_maps = _prep_inputs(x, H, _cfg)
    res = run_bass_kernel_spmd(nc, in_maps, core_ids=list(range(N_CORES)),
                               trace=_trace)
    out = _unscramble(res.results, _cfg)
    if _trace:
        return out, res
    return out
